# revision 25
# baseline (speedup 1.0000x reference)
"""Trainium2 Bass kernel for a single-head BERT attention (B=8, S=2048, E=1024, H=64).

Sharding: data-parallel over batch — one batch element per NeuronCore (8 cores).
Weights replicated. No collectives.

Design (all matmuls bf16 with fp32 PSUM accumulation):
  qkT   weight-stationary projection [Wq|Wk]: psum -> qkT_sb [128, S]
        (rows 0-63 q^T, 64-127 k^T); kT_sb rows 0-63 hold the k^T copy
        (SBUF->SBUF DMA) so mm1 runs with K=64 at tile (0,0).
  v     x-stationary projection, natural [t, h] layout + ones column.
  mm1   chunk-major (s-chunks of 512): per j-pair one [128, 2, 512] psum
        unit; widths clipped to the mask's live range per tile.
  exp   split across ACT (scalar.activation Exp) and DVE (bit-hack:
        round(x*a+b) -> int16 == bf16 bits of 2^(x*0.125*log2e)); units
        containing masked blocks go to ACT, then a gpsimd mask-multiply.
  mm2   pt-stationary into a per-chunk [128, 4, 65] psum (col 64 = softmax
        denominator via the ones column of v); batched reciprocal +
        broadcast-multiply normalize, one y DMA per chunk (bf16).

Chunk c's mm2 + finalize are deferred into slab c+1's emission region so the
PE always has ready work while x still streams (8 half-slab DMAs on the sync
queue). Weights + mask + kT fixups ride the gpsimd (SWDGE) queue. Warm-up
matmuls bridge the preamble-to-first-data window so the PE HAM clock gate
releases early.
"""

import numpy as np
import ml_dtypes

import concourse.bass as bass  # noqa: F401  (import registers bass machinery)
import concourse.bacc as bacc
import concourse.mybir as mybir
import concourse.tile as tile
from concourse.bass_utils import run_bass_kernel_spmd

BF16 = ml_dtypes.bfloat16
B, S, E, H = 8, 2048, 1024, 64
P = 128          # partitions / tile edge
NS = S // P      # 16 seq tiles
NE = E // P      # 8 embed chunks
SLAB = 512       # s-slab width (one PSUM bank of fp32)
NSL = S // SLAB  # 4 slabs == 4 chunks
TPS = SLAB // P  # 4 t-tiles per slab
WARMUP_MMS = 14  # dummy matmuls to release the HAM clock gate early

# exp bit-hack constants: exp(x/8) == 2^(x*0.125*log2e); bf16 bits of that
# are round(x*EXPA + EXPB) as int16 (calibrated c=5.5 for round-to-nearest).
EXPA = float(0.125 * np.log2(np.e) * 128.0)
EXPB = float(127.0 * 128.0 - 5.5)
DVE_CLEAN_UNITS = 8    # of the 12 clean (mask-free) exp units, how many on DVE
MASK_ON_GPSIMD = True  # route post-exp mask multiplies to the idle gpsimd

_cache: dict = {}
last_results = None  # BassKernelResults of the most recent run (for test harness)


def _plan_from_mask(mask: np.ndarray):
    """Derive the static block plan from the actual mask input.

    Returns (ranges, mask_items, n_uniq, mm2_lists, maskT):
      ranges[j]     = (lo, hi) element range of s that t-tile j must compute (or None)
      mask_items    = ((j, i, slot)) 128x128 blocks needing an elementwise mask
                      multiply; slot indexes the deduped unique-block upload
      mm2_lists[i]  = t-tiles contributing to output s-tile i
    Valid for every batch element simultaneously (classifications reduced over batch).
    """
    m = np.asarray(mask, dtype=bool)
    mt = np.ascontiguousarray(m.transpose(0, 2, 1))  # [B, t, s]
    blocks = mt.reshape(B, NS, P, NS, P)
    any_ = blocks.any(axis=(2, 4))   # [B, tj, si]
    all_ = blocks.all(axis=(2, 4))
    nz = any_.any(axis=0)            # not all-zero in some batch -> must compute
    allone = all_.all(axis=0)        # all-ones in every batch -> no mask needed
    mixed = nz & ~allone

    ranges = []
    for j in range(NS):
        cols = np.nonzero(nz[j])[0]
        if len(cols) == 0:
            ranges.append(None)
            continue
        ranges.append((int(cols.min()) * P, (int(cols.max()) + 1) * P))

    uniq = {}
    mask_items = []
    for j in range(NS):
        for i in range(NS):
            if not mixed[j, i]:
                continue
            key = blocks[:, j, :, i, :].tobytes()
            slot = uniq.setdefault(key, len(uniq))
            mask_items.append((j, i, slot))
    mm2 = [tuple(int(j) for j in np.nonzero(nz[:, i])[0]) for i in range(NS)]
    return ranges, tuple(mask_items), len(uniq), mm2, mt


def _build_nc(ranges, mask_items, n_uniq, mm2, has_bqk, has_bv):
    dt = mybir.dt
    n_mb = max(n_uniq, 1)
    nc = bacc.Bacc("TRN2", target_bir_lowering=False, debug=False, num_devices=8)

    EXP = mybir.ActivationFunctionType.Exp
    MULT = mybir.AluOpType.mult
    ADD = mybir.AluOpType.add

    xq_d = nc.dram_tensor("xq", [P, NSL * NE * SLAB], dt.bfloat16,
                          kind="ExternalInput").ap()
    wqk_d = nc.dram_tensor("wqk", [P, NE * 2 * H], dt.bfloat16, kind="ExternalInput").ap()
    wv_d = nc.dram_tensor("wv", [P, NE * H], dt.bfloat16, kind="ExternalInput").ap()
    mb_d = nc.dram_tensor("maskb", [P, n_mb * P], dt.bfloat16, kind="ExternalInput").ap()
    if has_bqk:
        bqk_d = nc.dram_tensor("bqk", [1, 2 * H], dt.bfloat16, kind="ExternalInput").ap()
    if has_bv:
        bv_d = nc.dram_tensor("bv", [1, H], dt.bfloat16, kind="ExternalInput").ap()
    y_d = nc.dram_tensor("y", [S, H], dt.bfloat16, kind="ExternalOutput").ap()
    y_t = y_d.rearrange("(i p) h -> p i h", p=P)

    # mm1 unit (p, c): tiles jA=2p, jB=2p+1, columns clipped to live ranges
    def tile_cols(j, c_):
        if ranges[j] is None:
            return None
        lo, hi = ranges[j]
        a, b_ = max(lo, c_ * SLAB), min(hi, (c_ + 1) * SLAB)
        return (a, b_) if a < b_ else None

    def unit_live(p_, c_):
        return tile_cols(2 * p_, c_) is not None or tile_cols(2 * p_ + 1, c_) is not None

    masked_units = set()
    for (j, i, slot) in mask_items:
        masked_units.add((j // 2, (i * P) // SLAB))

    clean_units = [(p_, c_) for c_ in range(NSL) for p_ in range(NS // 2)
                   if unit_live(p_, c_) and (p_, c_) not in masked_units]
    dve_units = set(clean_units[i] for i in
                    np.linspace(0, len(clean_units) - 1,
                                min(DVE_CLEAN_UNITS, len(clean_units)),
                                dtype=int)) if clean_units else set()

    with tile.TileContext(nc) as tc:
        with (
            tc.tile_pool(name="consts", bufs=1) as cpool,
            tc.tile_pool(name="xt", bufs=1) as xpool,
            tc.tile_pool(name="qk", bufs=1) as qkpool,
            tc.tile_pool(name="vex", bufs=1) as vpool,
            tc.tile_pool(name="pt", bufs=1) as ppool,
            tc.tile_pool(name="maskp", bufs=1) as mpool,
            tc.tile_pool(name="outs", bufs=2) as opool,
            tc.tile_pool(name="wps", bufs=2, space="PSUM") as wpsum,     # mm1 pairs (2 banks ea)
            tc.tile_pool(name="pjps", bufs=1, space="PSUM") as pjpool,   # qk proj (1 bank)
            tc.tile_pool(name="vps", bufs=1, space="PSUM") as vpsum,     # v proj (1 bank)
            tc.tile_pool(name="ctxps", bufs=2, space="PSUM") as cxpool,  # mm2 (1 bank ea)
        ):
            # ---- tiny constants (vector engine, before anything else) ----
            zeros_sb = cpool.tile([P, SLAB], dt.bfloat16)
            nc.vector.memset(zeros_sb[:], 0.0)

            # ---- input DMAs ----
            # x: 8 half-slab transfers on the sync (HWDGE) queue
            xq = xpool.tile([P, NSL, NE, SLAB], dt.bfloat16, name="xq")
            xqg = xq_d.rearrange("p (k c s) -> p k c s", k=NSL, c=NE)
            HC = NE // 2  # chunks per half-slab
            for k in range(NSL):
                for h in range(2):
                    nc.sync.dma_start(xq[:, k, h * HC:(h + 1) * HC],
                                      xqg[:, k, h * HC:(h + 1) * HC])
            # weights + mask on the gpsimd (SWDGE) queue
            wqk_sb = cpool.tile([P, NE, 2 * H], dt.bfloat16)
            nc.gpsimd.dma_start(wqk_sb[:], wqk_d.rearrange("p (c h) -> p c h", c=NE))
            wv_sb = cpool.tile([P, NE, H], dt.bfloat16)
            nc.gpsimd.dma_start(wv_sb[:], wv_d.rearrange("p (c h) -> p c h", c=NE))
            mask_all = mpool.tile([P, n_mb * P], dt.bfloat16, name="mask_all")
            nc.gpsimd.dma_start(mask_all[:], mb_d[:])
            mask_tiles = {}
            for (j, i, slot) in mask_items:
                mask_tiles[(j, i)] = mask_all[:, slot * P:(slot + 1) * P]
            ones_sb = cpool.tile([1, SLAB], dt.bfloat16)
            nc.vector.memset(ones_sb[:], 1.0)
            if has_bqk:
                bqk_sb = cpool.tile([1, 2 * H], dt.bfloat16)
                nc.gpsimd.dma_start(bqk_sb[:], bqk_d[:])
            if has_bv:
                bv_sb = cpool.tile([1, H], dt.bfloat16)
                nc.gpsimd.dma_start(bv_sb[:], bv_d[:])

            # ---- PE warm-up: contentless matmuls to release the HAM gate ----
            warm_ps = [wpsum.tile([P, 2, SLAB], dt.float32, tag="wps", name="wmps")
                       for _ in range(2)]
            for n in range(WARMUP_MMS):
                nc.tensor.matmul(warm_ps[n % 2][:, 0, :], zeros_sb[:, 0:P],
                                 zeros_sb[:], start=True, stop=True)

            # ---- persistent SBUF tensors ----
            qkT_sb = qkpool.tile([P, S], dt.bfloat16)   # rows 0-63 q^T, 64-127 k^T
            kT_sb = qkpool.tile([64, S], dt.bfloat16)   # k^T on partitions 0-63
            vall = vpool.tile([P, NS, H + 1], dt.bfloat16, name="vall")
            nc.vector.memset(vall[:, :, H], 1.0)        # ones column (denominator)
            pt = ppool.tile([P, NS, S], dt.bfloat16, name="pt_all")

            def emit_proj(k):
                """q/k projection for slab k -> qkT_sb cols, k^T fixup DMA."""
                cols = slice(k * SLAB, (k + 1) * SLAB)
                ps = pjpool.tile([P, SLAB], dt.float32, tag="pj", name=f"qkps{k}")
                for c in range(NE):
                    nc.tensor.matmul(ps[:], wqk_sb[:, c, :], xq[:, k, c, :],
                                     start=(c == 0),
                                     stop=(not has_bqk and c == NE - 1))
                if has_bqk:
                    nc.tensor.matmul(ps[:], bqk_sb[:], ones_sb[:],
                                     start=False, stop=True)
                nc.vector.tensor_copy(qkT_sb[:, cols], ps[:])
                nc.gpsimd.dma_start(kT_sb[:, cols], qkT_sb[64:128, cols])

            def emit_v(k):
                """v projection for slab k's four t-tiles (x-stationary)."""
                pv = vpsum.tile([P, TPS, H], dt.float32, tag="vps", name=f"pv{k}")
                for t in range(TPS):
                    for c in range(NE):
                        nc.tensor.matmul(
                            pv[:, t, :], xq[:, k, c, t * P:(t + 1) * P],
                            wv_sb[:, c, :],
                            start=(c == 0), stop=(not has_bv and c == NE - 1))
                    if has_bv:
                        nc.tensor.matmul(pv[:, t, :], ones_sb[:, 0:P], bv_sb[:],
                                         start=False, stop=True)
                nc.vector.tensor_copy(vall[:, 4 * k:4 * k + 4, 0:H], pv[:])

            def emit_mm1_pair(p_, c_, w2):
                """mm1 for tiles jA=2p, jB=2p+1 over chunk c (clipped widths)."""
                for row, j in ((0, 2 * p_), (1, 2 * p_ + 1)):
                    tc_ = tile_cols(j, c_)
                    if tc_ is None:
                        continue
                    a, b_ = tc_
                    rel = a - c_ * SLAB
                    nc.tensor.matmul(
                        w2[:, row, rel:rel + (b_ - a)],
                        kT_sb[:, j * P:(j + 1) * P],
                        qkT_sb[0:64, a:b_],
                        start=True, stop=True, tile_position=(0, 0))

            def emit_exp(p_, c_, w2):
                cols = slice(c_ * SLAB, (c_ + 1) * SLAB)
                j = 2 * p_
                dst = pt[:, j:j + 2, cols]
                if (p_, c_) in dve_units:
                    nc.vector.tensor_scalar(dst.bitcast(dt.int16), w2[:],
                                            EXPA, EXPB, MULT, ADD)
                else:
                    nc.scalar.activation(dst, w2[:], EXP, scale=0.125)
                # mask-multiply any mixed 128x128 block this unit produced
                for j_ in (j, j + 1):
                    for i_ in range(c_ * TPS, (c_ + 1) * TPS):
                        mt_ = mask_tiles.get((j_, i_))
                        if mt_ is not None:
                            sl = pt[:, j_, i_ * P:(i_ + 1) * P]
                            if MASK_ON_GPSIMD:
                                nc.gpsimd.tensor_mul(sl, sl, mt_)
                            else:
                                nc.vector.tensor_mul(sl, sl, mt_)

            started = {}

            def emit_mm2_pair(p_, c_, cx, key):
                """mm2 contributions of tiles 2p/2p+1 to chunk c's four outputs."""
                for i_ in range(c_ * TPS, (c_ + 1) * TPS):
                    for j_ in (2 * p_, 2 * p_ + 1):
                        if j_ not in mm2[i_]:
                            continue
                        st = not started.get(key, False)
                        started[key] = True
                        nc.tensor.matmul(
                            cx[:, i_ % TPS, :], pt[:, j_, i_ * P:(i_ + 1) * P],
                            vall[:, j_, :], start=st, stop=(j_ == max(mm2[i_])))

            def emit_fin(c_, cx):
                """Batched normalize + one y DMA for chunk c's four tiles."""
                ob = opool.tile([P, TPS, H], dt.bfloat16, tag="ob", name=f"ob{c_}")
                rc = opool.tile([P, TPS], dt.float32, tag="rc", name=f"rc{c_}")
                nc.vector.reciprocal(rc[:], cx[:, :, H])
                nc.vector.tensor_tensor(
                    ob[:], cx[:, :, 0:H],
                    rc[:].unsqueeze(2).broadcast_to([P, TPS, H]), MULT)
                for i_ in range(c_ * TPS, (c_ + 1) * TPS):
                    if not mm2[i_]:
                        nc.vector.memset(ob[:, i_ % TPS, :], 0.0)
                nc.sync.dma_start(y_t[:, c_ * TPS:(c_ + 1) * TPS, :], ob[:])

            # ---- main schedule ----
            # Region k: proj(k), v(k), then chunk k's mm1+exp pipeline with
            # chunk k-1's (deferred) mm2+fin interleaved as PE filler.
            cxs = {}
            for k in range(NSL):
                c_ = k
                pairs = [p_ for p_ in range(NS // 2) if unit_live(p_, c_)]
                cx = cxpool.tile([P, TPS, H + 1], dt.float32, tag="cx",
                                 name=f"ctx{c_}")
                cxs[c_] = cx
                # deferred work from chunk k-1: mm2 per pair + finalize
                deferred = []
                if k >= 1:
                    pc = k - 1
                    pcx = cxs[pc]
                    for q_ in [p_ for p_ in range(NS // 2) if unit_live(p_, pc)]:
                        deferred.append((lambda q=q_, pc=pc, pcx=pcx:
                                         emit_mm2_pair(q, pc, pcx, f"ctx{pc}")))
                    deferred.append((lambda pc=pc, pcx=pcx: emit_fin(pc, pcx)))

                # All but the last pair's mm2 (whose exp may still be in
                # flight) goes BEFORE the DMA-gated proj: it is ready PE work
                # that fills the slab-k wait so the HAM clock stays released.
                di = 0
                while di < max(0, len(deferred) - 2):
                    deferred[di]()
                    di += 1
                emit_proj(k)
                emit_v(k)
                w2s = {}
                for n, p_ in enumerate(pairs):
                    w2 = wpsum.tile([P, 2, SLAB], dt.float32, tag="wps", name="wps")
                    w2s[p_] = w2
                    emit_mm1_pair(p_, c_, w2)
                    if n >= 1:
                        q_ = pairs[n - 1]
                        emit_exp(q_, c_, w2s.pop(q_))
                    if di < len(deferred):
                        deferred[di]()
                        di += 1
                if pairs:
                    q_ = pairs[-1]
                    emit_exp(q_, c_, w2s.pop(q_))
                while di < len(deferred):
                    deferred[di]()
                    di += 1
                if k == NSL - 1:
                    for q_ in pairs:
                        emit_mm2_pair(q_, c_, cx, f"ctx{c_}")
                    emit_fin(c_, cx)

    nc.compile()
    return nc


def kernel(x, mask, Wq, bq, Wk, bk, Wv, bv, _trace=False, _trace_kwargs=None):
    global last_results
    x = np.asarray(x, dtype=np.float32)
    ranges, mask_items, n_uniq, mm2, maskT = _plan_from_mask(mask)

    has_bqk = bool(np.any(bq)) or bool(np.any(bk))
    has_bv = bool(np.any(bv))
    key = (tuple(ranges), mask_items, n_uniq, tuple(mm2), has_bqk, has_bv)
    nc = _cache.get(key)
    if nc is None:
        nc = _build_nc(ranges, mask_items, n_uniq, mm2, has_bqk, has_bv)
        _cache[key] = nc

    wqk = np.concatenate([np.asarray(Wq), np.asarray(Wk)], axis=1)
    wqk = np.ascontiguousarray(
        wqk.reshape(NE, P, 2 * H).transpose(1, 0, 2)).reshape(P, NE * 2 * H).astype(BF16)
    wv = np.ascontiguousarray(
        np.asarray(Wv).reshape(NE, P, H).transpose(1, 0, 2)).reshape(P, NE * H).astype(BF16)
    bqk = np.concatenate([np.asarray(bq), np.asarray(bk)])[None, :].astype(BF16)
    bvv = np.asarray(bv)[None, :].astype(BF16)

    in_maps = []
    for b in range(B):
        # [p, slab, chunk, s] so each half-slab is contiguous per partition
        xT_b = x[b].T.astype(BF16)                       # [E, S]
        xqb = np.ascontiguousarray(
            xT_b.reshape(NE, P, NSL, SLAB).transpose(1, 2, 0, 3)
        ).reshape(P, NSL * NE * SLAB)
        if mask_items:
            by_slot = {}
            for (j, i, slot) in mask_items:
                by_slot.setdefault(slot, (j, i))
            mb = np.concatenate([
                maskT[b, j * P:(j + 1) * P, i * P:(i + 1) * P]
                for slot, (j, i) in sorted(by_slot.items())], axis=1).astype(BF16)
        else:
            mb = np.zeros((P, P), dtype=BF16)
        im = {"xq": xqb, "wqk": wqk, "wv": wv, "maskb": mb}
        if has_bqk:
            im["bqk"] = bqk
        if has_bv:
            im["bv"] = bvv
        in_maps.append(im)

    res = run_bass_kernel_spmd(
        nc, in_maps, core_ids=list(range(B)),
        trace=_trace, **(_trace_kwargs or {}))
    last_results = res
    return np.stack([res.results[b]["y"].astype(np.float32) for b in range(B)])
